# revision 24
# baseline (speedup 1.0000x reference)
"""Trainium2 Bass kernel for the 2-layer CIN — square-trick + DMA-replication.

Reference computation (per batch element b, embedding channel d):
  z0[hf=h*40+f]  = x[b,h,d] * x[b,f,d]              (h,f in 0..39)
  y0[o]          = relu(sum_hf W0[o,hf,d] * z0[hf] + b0[o])   -> x1[b,o,d]
  z1[hf=h1*40+f] = x1[b,h1,d] * x[b,f,d]            (h1 in 0..63)
  y1[o]          = relu(sum_hf W1[o,hf,d] * z1[hf] + b1[o])   -> x2[b,o,d]
  out[b] = [sum_d x[b,:,d] | sum_d x1[b,:,d] | sum_d x2[b,:,d]]   (2048, 168)

Engine balance (per d-group of 2 channels x 512 batch = 1024 columns):

- Layer 1 rides the square trick x_h*x_f = ((x_h+x_f)^2 - x_h^2 - x_f^2)/2.
  The 820 unordered pairs PLUS 40 one-hot "correction" rows (whose squared
  selection is x_h^2 against the folded negative weight sums) pack into
  7x128 K-tiles: one 2-hot selection matmul on PE (fp32 PSUM), Square on
  ScalarE, weight matmul on PE.  Merging the corrections into the tiles
  removes the separate correction matmuls and the static x^2 tensor.
- Layer 2 tiles are ALL DMA-replicated products: tile t covers
  f in {2t,2t+1} x h1 in 0..63; xh = x[f] broadcast straight from DRAM xt
  (stride-0 read), multiplied on DVE (bf16 2x mode) against a resident
  x1rep whose two partition halves are both written by ScalarE relu
  activations (partition-offset writes; no duplication DMA).  This drops
  the layer-2 selection matmuls, corrections, and x1^2 products entirely.
- d-sum accumulations run on Pool; relu(y1+b1) on ScalarE; output
  transposes on PE.  Sharding: 4-way batch x 2-way embedding-channel
  split (8 cores), host adds the two d-halves.
"""

from contextlib import ExitStack

import numpy as np
import ml_dtypes

import concourse.bass as bass
import concourse.bacc as bacc
import concourse.tile as tile
from concourse import mybir
from concourse.bass_utils import run_bass_kernel_spmd
from concourse.masks import make_identity

BF16 = mybir.dt.bfloat16
FP32 = mybir.dt.float32
NPBF16 = ml_dtypes.bfloat16
ACT = mybir.ActivationFunctionType

B, F, D = 2048, 40, 32
O0, O1 = 64, 64
NCORES = 8
NB = 4                      # batch shards
ND = 2                      # d shards
BC = B // NB                # 512 batch rows per core
DC = D // ND                # 16 embedding channels per core
KT = 128
NP0 = (F * (F + 1)) // 2    # 820 unordered layer-1 pairs
NR0 = NP0 + F               # 860 layer-1 K-rows incl. merged corrections
NT0 = (NR0 + KT - 1) // KT  # 7 K-tiles, layer 1 (all V1)
NT1 = (O0 * F) // KT        # 20 f-major K-tiles, layer 2 (all V4)
DPG = 2                     # d-channels per group (one PSUM pair-tile)
DG = DC // DPG              # 8 d-groups
NCOL = DPG * BC             # 1024 free columns per group (d-major, b-minor)
NMM = 512                   # fp32-PSUM matmul free size
X1O = 64                    # x1 duplicate offset within x1rep

L1_PAIRS = [(h, f) for h in range(F) for f in range(h, F)]
assert len(L1_PAIRS) == NP0


def _build_bass(reps=1):
    nc = bacc.Bacc()
    xt = nc.declare_dram_parameter("xt", [F, DC * BC], BF16, isOutput=False)
    w0t = nc.declare_dram_parameter("w0t", [KT, NT0 * DC * O0], BF16, isOutput=False)
    w1t = nc.declare_dram_parameter("w1t", [KT, NT1 * DC * O1], BF16, isOutput=False)
    sel0 = nc.declare_dram_parameter("sel0", [F, NT0 * KT], BF16, isOutput=False)
    b0 = nc.declare_dram_parameter("b0", [O0, 1], FP32, isOutput=False)
    b1 = nc.declare_dram_parameter("b1", [O1, 1], FP32, isOutput=False)
    out = nc.declare_dram_parameter("out", [BC, O0 + O1], FP32, isOutput=True)

    with ExitStack() as ctx:
        tc = ctx.enter_context(tile.TileContext(nc))
        singles = ctx.enter_context(tc.tile_pool(name="singles", bufs=1))
        su_ps = ctx.enter_context(tc.tile_pool(name="su_ps", bufs=4, space="PSUM"))
        y_ps = ctx.enter_context(tc.tile_pool(name="y_ps", bufs=4, space="PSUM"))
        z_sb = ctx.enter_context(tc.tile_pool(name="z_sb", bufs=4))
        z2_sb = ctx.enter_context(tc.tile_pool(name="z2_sb", bufs=8))
        xh_sb = ctx.enter_context(tc.tile_pool(name="xh_sb", bufs=8))
        x2_sb = ctx.enter_context(tc.tile_pool(name="x2_sb", bufs=2))
        o_sb = ctx.enter_context(tc.tile_pool(name="o_sb", bufs=2))

        # ---- resident tensors ----
        xstack = singles.tile([F, DC * BC], BF16)
        x1rep = singles.tile([KT, DC * BC], BF16)   # x1 duplicated in halves
        w0s = singles.tile([KT, NT0, DC * O0], BF16)
        w1s = singles.tile([KT, NT1, DC * O1], BF16)
        sel0s = singles.tile([F, NT0, KT], BF16)
        b0s = singles.tile([O0, 1], FP32)
        b1s = singles.tile([O1, 1], FP32)

        xt_ap = xt[:]

        def load_inputs():
            nc.gpsimd.dma_start(out=sel0s, in_=sel0[:])
            nc.gpsimd.dma_start(out=xstack, in_=xt[:])
            nc.gpsimd.dma_start(out=b0s, in_=b0[:])
            nc.gpsimd.dma_start(out=b1s, in_=b1[:])
            nc.sync.dma_start(out=w0s, in_=w0t[:])
            nc.gpsimd.dma_start(out=w1s, in_=w1t[:])

        ident = singles.tile([128, 128], FP32)
        make_identity(nc, ident)

        # split accumulators (even/odd groups) halve the serial
        # read-modify-write chain on Pool; merged once in the epilogue
        acc1 = singles.tile([O0, BC], FP32)
        acc2 = singles.tile([O1, BC], FP32)
        acc1b = singles.tile([O0, BC], FP32)
        acc2b = singles.tile([O1, BC], FP32)

        def l1_tile(g, t, yp):
            """One layer-1 V1 tile: selection matmul, Square, weight matmul.

            Processed in two 512-column (one-d-channel) chunks so the PSUM
            staging rotates at half-tile granularity and the weight matmul
            for chunk i only waits on chunk i's square."""
            col0 = g * NCOL
            z = z_sb.tile([KT, NCOL], BF16, tag="z")
            for i in range(DPG):
                d = g * DPG + i
                su = su_ps.tile([KT, NMM], FP32, tag="su")
                nc.tensor.matmul(
                    su,
                    lhsT=sel0s[0:F, t, :],
                    rhs=xstack[0:F, col0 + i * NMM: col0 + (i + 1) * NMM],
                    start=True,
                    stop=True,
                )
                zc = z[:, i * BC:(i + 1) * BC]
                nc.scalar.activation(out=zc, in_=su, func=ACT.Square)
                nc.tensor.matmul(
                    yp[i * O0:(i + 1) * O0, :],
                    lhsT=w0s[:, t, d * O0:(d + 1) * O0],
                    rhs=zc,
                    start=(t == 0),
                    stop=(t == NT0 - 1),
                    skip_group_check=True,
                )

        def layer1(g):
            yp = y_ps.tile([2 * O0, BC], FP32, tag="y", name=f"y0_{g}")
            for t in range(NT0):
                l1_tile(g, t, yp)
            return yp

        def memset_accs():
            nc.gpsimd.memset(acc1, 0.0)
            nc.gpsimd.memset(acc2, 0.0)
            nc.gpsimd.memset(acc1b, 0.0)
            nc.gpsimd.memset(acc2b, 0.0)

        load_inputs()
        memset_accs()
        yp0 = layer1(0)
        for rep in range(reps):
          for g in range(DG):
            col0 = g * NCOL

            def relu_x1(i):
                # relu(y0 + b0) written to BOTH partition halves of x1rep by
                # ScalarE (partition-offset writes), so no duplication DMA.
                for lo in (X1O, 0):
                    nc.scalar.activation(
                        out=x1rep[lo:lo + O0, col0 + i * BC: col0 + (i + 1) * BC],
                        in_=yp0[i * O0:(i + 1) * O0, :],
                        func=ACT.Relu,
                        bias=b0s,
                        scale=1.0,
                    )

            def xh2_src(t):
                return bass.AP(
                    tensor=xt_ap.tensor,
                    offset=xt_ap.offset + 2 * t * DC * BC + col0,
                    ap=[[DC * BC, 2], [0, O0], [1, NCOL]],
                )

            def l2_tile(t, yp1):
                xh = xh_sb.tile([KT, NCOL], BF16, tag="xh")
                nc.sync.dma_start(out=xh, in_=xh2_src(t))
                z = z2_sb.tile([KT, NCOL], BF16, tag="z2")
                if t < 2:
                    # per-d-chunk multiply: chunk 0 only needs the first
                    # pair of relu writes, so DVE starts ~1.2us earlier
                    for i in range(DPG):
                        nc.vector.tensor_mul(
                            z[:, i * BC:(i + 1) * BC],
                            xh[:, i * BC:(i + 1) * BC],
                            x1rep[:, col0 + i * BC: col0 + (i + 1) * BC],
                        )
                else:
                    nc.vector.tensor_mul(z, xh, x1rep[:, col0:col0 + NCOL])
                for i in range(DPG):
                    d = g * DPG + i
                    nc.tensor.matmul(
                        yp1[i * O1:(i + 1) * O1, :],
                        lhsT=w1s[:, t, d * O1:(d + 1) * O1],
                        rhs=z[:, i * BC:(i + 1) * BC],
                        start=(t == 0),
                        stop=(t == NT1 - 1),
                        skip_group_check=True,
                    )

            # software pipeline across groups AND reps: the next group's
            # layer-1 tiles (which read only static x rows) are interleaved
            # with this group's layer-2 tiles so the PE queue reaches each
            # y0 matmul only after its square had time to complete, and the
            # ScalarE queue alternates squares with the relu writes.
            has_next = (g + 1 < DG) or (rep + 1 < reps)
            next_g = (g + 1) % DG
            yp0_next = (
                y_ps.tile([2 * O0, BC], FP32, tag="y", name=f"y0_{next_g}")
                if has_next else None
            )
            yp1 = y_ps.tile([2 * O1, BC], FP32, tag="y", name=f"y1_{g}")

            if has_next:
                l1_tile(next_g, 0, yp0_next)
            relu_x1(0)
            if has_next:
                l1_tile(next_g, 1, yp0_next)
            relu_x1(1)
            for i in range(DPG):
                a1 = acc1 if g % 2 == 0 else acc1b
                nc.gpsimd.tensor_add(
                    a1, a1, x1rep[0:O0, col0 + i * BC: col0 + (i + 1) * BC]
                )
            if has_next:
                for lt in range(2, NT0):
                    l1_tile(next_g, lt, yp0_next)
            for t in range(NT1):
                l2_tile(t, yp1)
            for i in range(DPG):
                x2 = x2_sb.tile([O1, BC], BF16, tag="x2")
                nc.scalar.activation(
                    out=x2,
                    in_=yp1[i * O1:(i + 1) * O1, :],
                    func=ACT.Relu,
                    bias=b1s,
                    scale=1.0,
                )
                a2 = acc2 if g % 2 == 0 else acc2b
                nc.gpsimd.tensor_add(a2, a2, x2)
            yp0 = yp0_next

          # ---- epilogue: accumulating transposes merge the split
          # accumulators in PSUM (no Pool merge adds), then store.
          # The even-half accumulators (acc1/acc2) are final after group
          # DG-2, so their transposes were already issued above; here the
          # odd halves close each accumulation group. ----
          for bh in range(BC // 128):
            outT = o_sb.tile([128, O0 + O1], FP32, tag="outT")
            for accs_, off in (((acc1, acc1b), 0), ((acc2, acc2b), O0)):
                pt = pt_tiles[(bh, off)]
                nc.tensor.matmul(
                    pt,
                    lhsT=accs_[1][:, bh * 128:(bh + 1) * 128],
                    rhs=ident[0:64, 0:64],
                    is_transpose=True,
                    start=False,
                    stop=True,
                    skip_group_check=True,
                )
                nc.vector.tensor_copy(out=outT[:, off:off + 64], in_=pt)
            nc.sync.dma_start(
                out=out[bh * 128:(bh + 1) * 128, :], in_=outT
            )
          if rep + 1 < reps:
            memset_accs()

    nc.compile()
    return nc


_NC_CACHE = {}
LAST_RESULT = None


def _get_nc(reps=1):
    if reps not in _NC_CACHE:
        _NC_CACHE[reps] = _build_bass(reps)
    return _NC_CACHE[reps]


def _l2_pair(t, p):
    """f-major layer-2 packing: tile t, row p -> (h1, f, hf)."""
    f = 2 * t + p // O0
    h1 = p % O0
    return h1, f, h1 * F + f


def _host_prep(x, W0, b0, W1, b1):
    """Build per-core input maps (host-side layout prep, all cheap numpy)."""
    def prep_w0(dh):
        Wd = W0[:, :, dh * DC:(dh + 1) * DC].astype(np.float32)  # (o, 1600, DC)
        # folded pair weights and one-hot correction weights share the tiles
        corr = np.zeros((F, O0, DC), dtype=np.float32)
        rows = np.zeros((NR0, O0, DC), dtype=np.float32)
        for r, (h, f) in enumerate(L1_PAIRS):
            w = Wd[:, h * F + f, :]
            if f != h:
                w = w + Wd[:, f * F + h, :]
            rows[r] = 0.5 * w
            corr[h] -= 0.5 * w
            corr[f] -= 0.5 * w
        rows[NP0:] = corr
        tiles = np.zeros((NT0, KT, DC * O0), dtype=NPBF16)
        for t in range(NT0):
            blk = rows[t * KT:(t + 1) * KT]          # (rows, O0, DC)
            tiles[t, :blk.shape[0]] = (
                blk.transpose(0, 2, 1).reshape(blk.shape[0], DC * O0)
                .astype(NPBF16)
            )
        return np.ascontiguousarray(
            tiles.transpose(1, 0, 2).reshape(KT, NT0 * DC * O0)
        )

    def prep_w1(dh):
        Wd = W1[:, :, dh * DC:(dh + 1) * DC].astype(np.float32)  # (o, 2560, DC)
        tiles = np.zeros((NT1, KT, DC * O1), dtype=NPBF16)
        for t in range(NT1):
            blk = np.zeros((O1, KT, DC), dtype=np.float32)
            for p in range(KT):
                _, _, hf = _l2_pair(t, p)
                blk[:, p, :] = Wd[:, hf, :]
            tiles[t] = (
                blk.transpose(1, 2, 0).reshape(KT, DC * O1).astype(NPBF16)
            )
        return np.ascontiguousarray(
            tiles.transpose(1, 0, 2).reshape(KT, NT1 * DC * O1)
        )

    sel0v = np.zeros((F, NT0, KT), dtype=np.float32)
    for r, (h, f) in enumerate(L1_PAIRS):
        sel0v[h, r // KT, r % KT] += 1.0
        sel0v[f, r // KT, r % KT] += 1.0
    for h in range(F):
        r = NP0 + h
        sel0v[h, r // KT, r % KT] = 1.0
    sel0v = sel0v.reshape(F, NT0 * KT).astype(NPBF16)

    b0h = b0.reshape(O0, 1).astype(np.float32)
    b1h = b1.reshape(O1, 1).astype(np.float32)

    halves = []
    for dh in range(ND):
        halves.append({
            "w0t": prep_w0(dh),
            "w1t": prep_w1(dh),
        })

    in_maps = []
    for c in range(NCORES):
        bs, dh = c % NB, c // NB
        xc = x[bs * BC:(bs + 1) * BC]                    # (512, 40, 32)
        xtc = np.ascontiguousarray(
            xc[:, :, dh * DC:(dh + 1) * DC].transpose(1, 2, 0).reshape(F, DC * BC)
        ).astype(NPBF16)
        in_maps.append({
            "xt": xtc,
            "sel0": sel0v,
            "b0": b0h,
            "b1": b1h,
            **halves[dh],
        })
    return in_maps


def kernel(x, W0, b0, W1, b1):
    global LAST_RESULT
    x = np.asarray(x, dtype=np.float32)
    W0 = np.asarray(W0, dtype=np.float32)
    W1 = np.asarray(W1, dtype=np.float32)
    b0 = np.asarray(b0, dtype=np.float32)
    b1 = np.asarray(b1, dtype=np.float32)

    nc = _get_nc()
    in_maps = _host_prep(x, W0, b0, W1, b1)
    res = run_bass_kernel_spmd(nc, in_maps, core_ids=list(range(NCORES)))
    LAST_RESULT = res

    out = np.empty((B, F + O0 + O1), dtype=np.float32)
    out[:, :F] = x.sum(axis=-1)
    for bs in range(NB):
        half0 = np.asarray(res.results[bs]["out"])
        half1 = np.asarray(res.results[NB + bs]["out"])
        out[bs * BC:(bs + 1) * BC, F:] = half0 + half1
    return out


# revision 36
# speedup vs baseline: 9.1638x; 9.1638x over previous
"""Trainium2 Bass kernel for the 2-layer CIN — square-trick + streamed products.

Reference computation (per batch element b, embedding channel d):
  z0[hf=h*40+f]  = x[b,h,d] * x[b,f,d]              (h,f in 0..39)
  y0[o]          = relu(sum_hf W0[o,hf,d] * z0[hf] + b0[o])   -> x1[b,o,d]
  z1[hf=h1*40+f] = x1[b,h1,d] * x[b,f,d]            (h1 in 0..63)
  y1[o]          = relu(sum_hf W1[o,hf,d] * z1[hf] + b1[o])   -> x2[b,o,d]
  out[b] = [sum_d x[b,:,d] | sum_d x1[b,:,d] | sum_d x2[b,:,d]]   (2048, 168)

Hardware findings that shape this kernel (measured via ablation probes):
stride-0 broadcast-gather DMAs from HBM run ~7x slower than the cost
model (~4.7us per [128,1024] tile), while contiguous HBM streams and
partition-aligned SBUF-SBUF copies run near the bandwidth model.  So the
layer-2 product tiles z1 = x1[h1]*x[f] are produced two ways:

- NSTREAM tiles stream a HOST-PREREPLICATED x[f] tensor (xhs, contiguous
  2KB/partition rows) from DRAM and multiply on DVE (bf16 2x mode)
  against a resident x1rep whose halves ScalarE relu writes directly.
- The remaining tiles use the square trick
  x1_h*x_f = ((x1_h+x_f)^2 - x1_h^2 - x_f^2)/2: a 2-hot selection matmul
  on PE over a resident [x | x1] stack, Square on DVE/Pool, then the
  0.5-folded weight matmul.  All corrections (including layer 1's) ride
  as 1-hot selection rows whose square IS x^2 against negated folded
  weight sums — no separate correction matmuls, no x^2 tensors.

Layer 1 is fully square-trick: 820 unordered pairs + 40 one-hot
correction rows pack into 7x128 K-tiles (squares on ScalarE, chunked per
512 columns so PSUM staging rotates at half-tile granularity).  The
layer-2 V1 tiles carry their corrections in a 21st tile (one-hot rows
over x_f and x1_h1).  d-sums accumulate on Pool into split even/odd
accumulators merged by accumulating PE transposes in the epilogue.
Sharding: 4-way batch x 2-way embedding-channel split (8 cores), host
adds the two d-halves.
"""

import os
from contextlib import ExitStack

import numpy as np
import ml_dtypes

import concourse.bass as bass
import concourse.bacc as bacc
import concourse.tile as tile
from concourse import mybir
from concourse.bass_utils import run_bass_kernel_spmd
from concourse.masks import make_identity

BF16 = mybir.dt.bfloat16
FP32 = mybir.dt.float32
NPBF16 = ml_dtypes.bfloat16
ACT = mybir.ActivationFunctionType

B, F, D = 2048, 40, 32
O0, O1 = 64, 64
NCORES = 8
NB = 4                      # batch shards
ND = 2                      # d shards
BC = B // NB                # 512 batch rows per core
DC = D // ND                # 16 embedding channels per core
KT = 128
NP0 = (F * (F + 1)) // 2    # 820 unordered layer-1 pairs
NR0 = NP0 + F               # 860 layer-1 K-rows incl. merged corrections
NT0 = (NR0 + KT - 1) // KT  # 7 K-tiles, layer 1 (all square-trick)
NT1 = (O0 * F) // KT        # 20 f-major K-tiles, layer 2
NSTREAM = int(os.environ.get("CIN_NSTREAM", "16"))  # streamed layer-2 tiles
NV1 = NT1 - NSTREAM         # layer-2 square-trick tiles
NCORR = 1 if NV1 > 0 else 0  # corr tile carrying x^2 / x1^2 one-hot rows
DPG = 2                     # d-channels per group (one PSUM pair-tile)
DG = DC // DPG              # 8 d-groups
NCOL = DPG * BC             # 1024 free columns per group (d-major, b-minor)
NMM = 512                   # fp32-PSUM matmul free size
X1O = 64                    # x1 row offset in the [x | x1] stack

L1_PAIRS = [(h, f) for h in range(F) for f in range(h, F)]
assert len(L1_PAIRS) == NP0


def _l2_pair(t, p):
    """f-major layer-2 packing: tile t, row p -> (h1, f, hf)."""
    f = 2 * t + p // O0
    h1 = p % O0
    return h1, f, h1 * F + f


def _build_bass(reps=1):
    nc = bacc.Bacc()
    xt = nc.declare_dram_parameter("xt", [F, DC * BC], BF16, isOutput=False)
    xhs = nc.declare_dram_parameter(
        "xhs", [KT, NSTREAM * DC * BC], BF16, isOutput=False)
    w0t = nc.declare_dram_parameter("w0t", [KT, NT0 * DC * O0], BF16, isOutput=False)
    w1t = nc.declare_dram_parameter(
        "w1t", [KT, (NT1 + NCORR) * DC * O1], BF16, isOutput=False)
    sel0 = nc.declare_dram_parameter("sel0", [F, NT0 * KT], BF16, isOutput=False)
    sel1 = nc.declare_dram_parameter(
        "sel1", [KT, max(NV1 + NCORR, 1) * KT], BF16, isOutput=False)
    b0 = nc.declare_dram_parameter("b0", [O0, 1], FP32, isOutput=False)
    b1 = nc.declare_dram_parameter("b1", [O1, 1], FP32, isOutput=False)
    out = nc.declare_dram_parameter("out", [BC, O0 + O1], FP32, isOutput=True)

    with ExitStack() as ctx:
        tc = ctx.enter_context(tile.TileContext(nc))
        singles = ctx.enter_context(tc.tile_pool(name="singles", bufs=1))
        su_ps = ctx.enter_context(tc.tile_pool(name="su_ps", bufs=4, space="PSUM"))
        y_ps = ctx.enter_context(tc.tile_pool(name="y_ps", bufs=4, space="PSUM"))
        z_sb = ctx.enter_context(tc.tile_pool(name="z_sb", bufs=4))
        z2_sb = ctx.enter_context(tc.tile_pool(name="z2_sb", bufs=8))
        xh_sb = ctx.enter_context(tc.tile_pool(name="xh_sb", bufs=8))
        x2_sb = ctx.enter_context(tc.tile_pool(name="x2_sb", bufs=2))
        o_sb = ctx.enter_context(tc.tile_pool(name="o_sb", bufs=2))

        # ---- resident tensors ----
        xstack = singles.tile([KT, DC * BC], BF16)  # rows 0-39 x, 64-127 x1
        x1rep = singles.tile([KT, DC * BC], BF16)   # x1 duplicated in halves
        w0s = singles.tile([KT, NT0, DC * O0], BF16)
        w1s = singles.tile([KT, NT1 + NCORR, DC * O1], BF16)
        sel0s = singles.tile([F, NT0, KT], BF16)
        sel1s = singles.tile([KT, max(NV1 + NCORR, 1), KT], BF16)
        b0s = singles.tile([O0, 1], FP32)
        b1s = singles.tile([O1, 1], FP32)

        xhs_ap = xhs[:]

        def load_inputs():
            nc.gpsimd.dma_start(out=sel0s, in_=sel0[:])
            nc.gpsimd.dma_start(out=sel1s, in_=sel1[:])
            # engine partition windows must start at multiples of 32: clear
            # rows 32-63 first, then the x DMA overwrites rows 0-39
            nc.vector.memset(xstack[32:X1O, :], 0.0)
            nc.gpsimd.dma_start(out=xstack[0:F, :], in_=xt[:])
            nc.gpsimd.dma_start(out=b0s, in_=b0[:])
            nc.gpsimd.dma_start(out=b1s, in_=b1[:])
            nc.sync.dma_start(out=w0s, in_=w0t[:])
            nc.gpsimd.dma_start(out=w1s, in_=w1t[:])

        ident = singles.tile([128, 128], FP32)
        make_identity(nc, ident)

        # split accumulators (even/odd groups) halve the serial
        # read-modify-write chain on Pool; merged in the epilogue
        acc1 = singles.tile([O0, BC], FP32)
        acc2 = singles.tile([O1, BC], FP32)
        acc1b = singles.tile([O0, BC], FP32)
        acc2b = singles.tile([O1, BC], FP32)

        def l1_tile(g, t, yp):
            """One layer-1 square-trick tile, chunked per d-channel."""
            col0 = g * NCOL
            z = z_sb.tile([KT, NCOL], BF16, tag="z")
            for i in range(DPG):
                d = g * DPG + i
                su = su_ps.tile([KT, NMM], FP32, tag="su")
                nc.tensor.matmul(
                    su,
                    lhsT=sel0s[0:F, t, :],
                    rhs=xstack[0:F, col0 + i * NMM: col0 + (i + 1) * NMM],
                    start=True,
                    stop=True,
                )
                zc = z[:, i * BC:(i + 1) * BC]
                nc.scalar.activation(out=zc, in_=su, func=ACT.Square)
                nc.tensor.matmul(
                    yp[i * O0:(i + 1) * O0, :],
                    lhsT=w0s[:, t, d * O0:(d + 1) * O0],
                    rhs=zc,
                    start=(t == 0),
                    stop=(t == NT0 - 1),
                    skip_group_check=True,
                )

        def layer1(g):
            yp = y_ps.tile([2 * O0, BC], FP32, tag="y", name=f"y0_{g}")
            for t in range(NT0):
                l1_tile(g, t, yp)
            return yp

        def memset_accs():
            nc.gpsimd.memset(acc1, 0.0)
            nc.gpsimd.memset(acc2, 0.0)
            nc.gpsimd.memset(acc1b, 0.0)
            nc.gpsimd.memset(acc2b, 0.0)

        load_inputs()
        memset_accs()
        yp0 = layer1(0)
        for rep in range(reps):
          for g in range(DG):
            col0 = g * NCOL

            def relu_x1(i):
                # relu(y0 + b0) written to BOTH partition halves of x1rep by
                # DVE tensor_scalar (one PSUM input is legal; partition-offset
                # writes), keeping ScalarE free for the square chain.
                for lo in (X1O, 0):
                    nc.vector.tensor_scalar(
                        out=x1rep[lo:lo + O0, col0 + i * BC: col0 + (i + 1) * BC],
                        in0=yp0[i * O0:(i + 1) * O0, :],
                        scalar1=b0s[:],
                        scalar2=0.0,
                        op0=mybir.AluOpType.add,
                        op1=mybir.AluOpType.max,
                    )

            def l2_stream_tile(t, yp1):
                # host-prereplicated x[f] rows: plain contiguous DRAM stream
                xh = xh_sb.tile([KT, NCOL], BF16, tag="xh")
                nc.sync.dma_start(out=xh, in_=bass.AP(
                    tensor=xhs_ap.tensor,
                    offset=xhs_ap.offset + t * DC * BC + col0,
                    ap=[[NSTREAM * DC * BC, KT], [1, NCOL]],
                ))
                z = z2_sb.tile([KT, NCOL], BF16, tag="z2")
                if t < 2:
                    # per-d-chunk multiply: chunk 0 only needs the first
                    # pair of relu writes, so DVE starts ~1.2us earlier
                    for i in range(DPG):
                        nc.vector.tensor_mul(
                            z[:, i * BC:(i + 1) * BC],
                            xh[:, i * BC:(i + 1) * BC],
                            x1rep[:, col0 + i * BC: col0 + (i + 1) * BC],
                        )
                else:
                    nc.vector.tensor_mul(z, xh, x1rep[:, col0:col0 + NCOL])
                for i in range(DPG):
                    d = g * DPG + i
                    nc.tensor.matmul(
                        yp1[i * O1:(i + 1) * O1, :],
                        lhsT=w1s[:, t, d * O1:(d + 1) * O1],
                        rhs=z[:, i * BC:(i + 1) * BC],
                        start=(t == 0),
                        stop=False,
                        skip_group_check=True,
                    )

            def l2_v1_tile(t, yp1):
                # square-trick over the resident [x | x1] stack; squares on
                # DVE/Pool keep ScalarE free for layer-1 squares and relus
                z = z2_sb.tile([KT, NCOL], BF16, tag="z2")
                st = t - NSTREAM
                for i in range(DPG):
                    d = g * DPG + i
                    su = su_ps.tile([KT, NMM], FP32, tag="su")
                    nc.tensor.matmul(
                        su,
                        lhsT=sel1s[:, st, :],
                        rhs=xstack[:, col0 + i * NMM: col0 + (i + 1) * NMM],
                        start=True,
                        stop=True,
                    )
                    zc = z[:, i * BC:(i + 1) * BC]
                    # TensorTensor may read only ONE operand from PSUM, so
                    # squares can only come from ScalarE's activation
                    nc.scalar.activation(out=zc, in_=su, func=ACT.Square)
                    nc.tensor.matmul(
                        yp1[i * O1:(i + 1) * O1, :],
                        lhsT=w1s[:, t, d * O1:(d + 1) * O1],
                        rhs=zc,
                        start=False,
                        stop=(t == NT1 + NCORR - 1),
                        skip_group_check=True,
                    )

            # software pipeline across groups AND reps: the next group's
            # layer-1 tiles run while this group's layer 2 streams.
            has_next = (g + 1 < DG) or (rep + 1 < reps)
            next_g = (g + 1) % DG
            yp0_next = (
                y_ps.tile([2 * O0, BC], FP32, tag="y", name=f"y0_{next_g}")
                if has_next else None
            )
            yp1 = y_ps.tile([2 * O1, BC], FP32, tag="y", name=f"y1_{g}")

            if has_next:
                l1_tile(next_g, 0, yp0_next)
            relu_x1(0)
            if has_next:
                l1_tile(next_g, 1, yp0_next)
            relu_x1(1)
            # x1 into the [x | x1] stack for the layer-2 selection matmuls:
            # partition-aligned SBUF-SBUF copy (fast path)
            if NV1 > 0:
                nc.sync.dma_start(
                    out=xstack[X1O:KT, col0:col0 + NCOL],
                    in_=x1rep[X1O:KT, col0:col0 + NCOL],
                )
            for i in range(DPG):
                a1 = acc1 if g % 2 == 0 else acc1b
                nc.gpsimd.tensor_add(
                    a1, a1, x1rep[0:O0, col0 + i * BC: col0 + (i + 1) * BC]
                )
            if has_next:
                for lt in range(2, NT0):
                    l1_tile(next_g, lt, yp0_next)
            for t in range(NSTREAM):
                l2_stream_tile(t, yp1)
            for t in range(NSTREAM, NT1 + NCORR):
                l2_v1_tile(t, yp1)
            for i in range(DPG):
                x2 = x2_sb.tile([O1, BC], BF16, tag="x2")
                nc.vector.tensor_scalar(
                    out=x2,
                    in0=yp1[i * O1:(i + 1) * O1, :],
                    scalar1=b1s[:],
                    scalar2=0.0,
                    op0=mybir.AluOpType.add,
                    op1=mybir.AluOpType.max,
                )
                a2 = acc2 if g % 2 == 0 else acc2b
                nc.gpsimd.tensor_add(a2, a2, x2)
            yp0 = yp0_next

          # ---- epilogue: accumulating transposes merge the split
          # accumulators in PSUM (no Pool merge adds), then store ----
          for bh in range(BC // 128):
            outT = o_sb.tile([128, O0 + O1], FP32, tag="outT")
            for accs_, off in (((acc1, acc1b), 0), ((acc2, acc2b), O0)):
                pt = y_ps.tile([128, 64], FP32, tag="y")
                for j, acc in enumerate(accs_):
                    nc.tensor.matmul(
                        pt,
                        lhsT=acc[:, bh * 128:(bh + 1) * 128],
                        rhs=ident[0:64, 0:64],
                        is_transpose=True,
                        start=(j == 0),
                        stop=(j == 1),
                        skip_group_check=True,
                    )
                nc.vector.tensor_copy(out=outT[:, off:off + 64], in_=pt)
            nc.sync.dma_start(
                out=out[bh * 128:(bh + 1) * 128, :], in_=outT
            )
          if rep + 1 < reps:
            memset_accs()

    nc.compile()
    return nc


_NC_CACHE = {}
LAST_RESULT = None


def _get_nc(reps=1):
    if reps not in _NC_CACHE:
        _NC_CACHE[reps] = _build_bass(reps)
    return _NC_CACHE[reps]


def _host_prep(x, W0, b0, W1, b1):
    """Build per-core input maps (host-side layout prep, all cheap numpy)."""
    def prep_w0(dh):
        Wd = W0[:, :, dh * DC:(dh + 1) * DC].astype(np.float32)  # (o, 1600, DC)
        corr = np.zeros((F, O0, DC), dtype=np.float32)
        rows = np.zeros((NR0, O0, DC), dtype=np.float32)
        for r, (h, f) in enumerate(L1_PAIRS):
            w = Wd[:, h * F + f, :]
            if f != h:
                w = w + Wd[:, f * F + h, :]
            rows[r] = 0.5 * w
            corr[h] -= 0.5 * w
            corr[f] -= 0.5 * w
        rows[NP0:] = corr
        tiles = np.zeros((NT0, KT, DC * O0), dtype=NPBF16)
        for t in range(NT0):
            blk = rows[t * KT:(t + 1) * KT]          # (rows, O0, DC)
            tiles[t, :blk.shape[0]] = (
                blk.transpose(0, 2, 1).reshape(blk.shape[0], DC * O0)
                .astype(NPBF16)
            )
        return np.ascontiguousarray(
            tiles.transpose(1, 0, 2).reshape(KT, NT0 * DC * O0)
        )

    def prep_w1(dh):
        Wd = W1[:, :, dh * DC:(dh + 1) * DC].astype(np.float32)  # (o, 2560, DC)
        tiles = np.zeros((NT1 + NCORR, KT, DC * O1), dtype=NPBF16)
        corr = np.zeros((KT, O1, DC), dtype=np.float32)
        for t in range(NT1):
            scale = 1.0 if t < NSTREAM else 0.5
            blk = np.zeros((O1, KT, DC), dtype=np.float32)
            for p in range(KT):
                h1, f, hf = _l2_pair(t, p)
                w = Wd[:, hf, :]
                blk[:, p, :] = scale * w
                if t >= NSTREAM:
                    corr[f] -= 0.5 * w
                    corr[X1O + h1] -= 0.5 * w
            tiles[t] = (
                blk.transpose(1, 2, 0).reshape(KT, DC * O1).astype(NPBF16)
            )
        if NCORR:
            tiles[NT1] = (
                corr.transpose(0, 2, 1).reshape(KT, DC * O1).astype(NPBF16)
            )
        return np.ascontiguousarray(
            tiles.transpose(1, 0, 2).reshape(KT, (NT1 + NCORR) * DC * O1)
        )

    sel0v = np.zeros((F, NT0, KT), dtype=np.float32)
    for r, (h, f) in enumerate(L1_PAIRS):
        sel0v[h, r // KT, r % KT] += 1.0
        sel0v[f, r // KT, r % KT] += 1.0
    for h in range(F):
        r = NP0 + h
        sel0v[h, r // KT, r % KT] = 1.0
    sel0v = sel0v.reshape(F, NT0 * KT).astype(NPBF16)

    sel1v = np.zeros((KT, max(NV1 + NCORR, 1), KT), dtype=np.float32)
    for t in range(NSTREAM, NT1):
        st = t - NSTREAM
        for p in range(KT):
            h1, f, _ = _l2_pair(t, p)
            sel1v[f, st, p] += 1.0
            sel1v[X1O + h1, st, p] += 1.0
    if NCORR:
        for f in range(F):
            sel1v[f, NV1, f] = 1.0
        for h1 in range(O1):
            sel1v[X1O + h1, NV1, X1O + h1] = 1.0
    sel1v = sel1v.reshape(KT, max(NV1 + NCORR, 1) * KT).astype(NPBF16)

    b0h = b0.reshape(O0, 1).astype(np.float32)
    b1h = b1.reshape(O1, 1).astype(np.float32)

    halves = []
    for dh in range(ND):
        halves.append({
            "w0t": prep_w0(dh),
            "w1t": prep_w1(dh),
        })

    # prereplicated x[f] stream rows: xhs[p, t, :] = xt[2t + p//64, :]
    f_idx = np.empty((KT, NSTREAM), dtype=np.int64)
    for p in range(KT):
        for t in range(NSTREAM):
            f_idx[p, t] = 2 * t + p // O0

    in_maps = []
    for c in range(NCORES):
        bs, dh = c % NB, c // NB
        xc = x[bs * BC:(bs + 1) * BC]                    # (512, 40, 32)
        xtc = np.ascontiguousarray(
            xc[:, :, dh * DC:(dh + 1) * DC].transpose(1, 2, 0).reshape(F, DC * BC)
        ).astype(NPBF16)
        xhsc = np.ascontiguousarray(
            xtc[f_idx].reshape(KT, NSTREAM * DC * BC)
        )
        in_maps.append({
            "xt": xtc,
            "xhs": xhsc,
            "sel0": sel0v,
            "sel1": sel1v,
            "b0": b0h,
            "b1": b1h,
            **halves[dh],
        })
    return in_maps


def kernel(x, W0, b0, W1, b1):
    global LAST_RESULT
    x = np.asarray(x, dtype=np.float32)
    W0 = np.asarray(W0, dtype=np.float32)
    W1 = np.asarray(W1, dtype=np.float32)
    b0 = np.asarray(b0, dtype=np.float32)
    b1 = np.asarray(b1, dtype=np.float32)

    nc = _get_nc()
    in_maps = _host_prep(x, W0, b0, W1, b1)
    res = run_bass_kernel_spmd(nc, in_maps, core_ids=list(range(NCORES)))
    LAST_RESULT = res

    out = np.empty((B, F + O0 + O1), dtype=np.float32)
    out[:, :F] = x.sum(axis=-1)
    for bs in range(NB):
        half0 = np.asarray(res.results[bs]["out"])
        half1 = np.asarray(res.results[NB + bs]["out"])
        out[bs * BC:(bs + 1) * BC, F:] = half0 + half1
    return out
